# revision 5
# baseline (speedup 1.0000x reference)
"""ColorNet (15x conv+BN+ReLU, 224x224) on 8 TRN2 NeuronCores.

Strategy: spatial H-sharding, 28 owned rows/core plus a 6-row shrinking
halo (one row per 3x3 conv) so no activation halo exchange is ever
needed; only BN batch-stats cross the cores (one small AllReduce per
layer).  Convs run as fp32r matmuls (full PE rate, ~1e-4 rel err)
accumulating 9 taps x Cin-blocks in PSUM; activations stream through
DRAM between layers with BN+ReLU fused into the next layer's load.
"""
import sys

sys.path.insert(0, "/opt/trn_rl_repo")

from contextlib import ExitStack

import numpy as np

import concourse.bacc as bacc
import concourse.tile as tile
from concourse import mybir
from concourse.bass_utils import run_bass_kernel_spmd

F32 = mybir.dt.float32
F32R = mybir.dt.float32r
AF = mybir.ActivationFunctionType

LAYERS = [(1, 64, 3), (64, 128, 1), (128, 128, 3),
          (128, 256, 1), (256, 256, 3), (256, 256, 1), (256, 256, 3),
          (256, 512, 1), (512, 512, 3), (512, 512, 1), (512, 512, 3),
          (512, 256, 1), (256, 128, 1), (128, 64, 1), (64, 3, 1)]
EPS = 1e-5
H = W = 224
NCORE = 8
OWN = 28            # owned output rows per core
RT = 40             # extended tile rows (6 + 28 + 6)
COLS = 226          # padded row width (1 + 224 + 1)
GP = 4              # row-pairs per load group
NPIX = 452          # matmul moving free size = 2 rows x 226
INTILE = 2 * GP * COLS + 2 * COLS + 2   # load-tile elems (GP pairs + halo + guards)
LUMW = (0.2125, 0.7154, 0.0721)


def _blocks(c):
    if c <= 128:
        return 1, c
    assert c % 128 == 0
    return c // 128, 128


def _plan(n_layers):
    plans = []
    m = 6
    for l in range(n_layers):
        cin, cout, k = LAYERS[l]
        m_in, m_out = m, m - (1 if k == 3 else 0)
        m = m_out
        n_icb, cib = _blocks(cin)
        n_ocb, cob = _blocks(cout)
        plans.append(dict(l=l, k=k, cin=cin, cout=cout, taps=k * k,
                          m_in=m_in, m_out=m_out, n_icb=n_icb, cib=cib,
                          n_ocb=n_ocb, cob=cob, o_start=6 - m_out,
                          o_rows=OWN + 2 * m_out))
    return plans


def build_program(n_layers):
    plans = _plan(n_layers)
    nc = bacc.Bacc(num_devices=NCORE)

    x9_in = nc.declare_dram_parameter("x9", [9, RT * COLS], F32, isOutput=False)
    w_in, p_in = [], []
    for pl in plans:
        l = pl["l"]
        wshape = ([1, 9, 1, 64] if l == 0 else
                  [pl["n_ocb"], pl["cib"], pl["n_icb"] * pl["taps"], pl["cob"]])
        w_in.append(nc.declare_dram_parameter(f"w{l}", wshape, F32, isOutput=False))
        p_in.append(nc.declare_dram_parameter(f"p{l}", [pl["n_ocb"], 128, 4], F32,
                                              isOutput=False))
    mask_in = nc.declare_dram_parameter("mask", [128, 2], F32, isOutput=False)
    lum_in = nc.declare_dram_parameter("lumw", [3, 1], F32, isOutput=False)
    last = plans[-1]
    full = n_layers == len(LAYERS)
    if full:
        xo = nc.declare_dram_parameter("xo", [3, OWN, W], F32, isOutput=True)
        yo = nc.declare_dram_parameter("yo", [1, OWN, W], F32, isOutput=True)
    else:  # debug build: dump last layer's raw (pre-BN) tile + its affine
        xo = nc.declare_dram_parameter(
            "xo", [last["n_ocb"], last["cob"], RT, COLS], F32, isOutput=True)
        yo = nc.declare_dram_parameter("yo", [128, 16], F32, isOutput=True)

    with tile.TileContext(nc) as tc, ExitStack() as ctx:
        pool_w = ctx.enter_context(tc.tile_pool(name="w", bufs=5))
        pool_in = ctx.enter_context(tc.tile_pool(name="in", bufs=6))
        pool_ev = ctx.enter_context(tc.tile_pool(name="ev", bufs=8))
        pool_ps = ctx.enter_context(tc.tile_pool(name="ps", bufs=8, space="PSUM"))
        pool_st = ctx.enter_context(tc.tile_pool(name="st", bufs=2))
        pool_rc = ctx.enter_context(tc.tile_pool(name="rc", bufs=6))
        pool_cn = ctx.enter_context(tc.tile_pool(name="cn", bufs=1))
        pool_dr = ctx.enter_context(tc.tile_pool(name="dr", bufs=2, space="DRAM"))
        pool_sh = ctx.enter_context(tc.tile_pool(name="sh", bufs=2, space="DRAM"))

        mask_sb = pool_cn.tile([128, 2], F32)
        nc.sync.dma_start(out=mask_sb, in_=mask_in[:])
        lum_sb = pool_cn.tile([3, 1], F32)
        nc.sync.dma_start(out=lum_sb.bitcast(F32R), in_=lum_in[:].bitcast(F32R))
        eps_sb = pool_cn.tile([128, 1], F32)
        nc.vector.memset(eps_sb, EPS)
        zero_sb = pool_cn.tile([128, RT], F32)
        nc.vector.memset(zero_sb, 0.0)

        act_prev = None          # DRAM tile holding previous layer's raw output
        a_prev = b_prev = None   # BN affine of previous layer's output

        for pl in plans:
            l, k, taps = pl["l"], pl["k"], pl["taps"]
            n_icb, cib = pl["n_icb"], pl["cib"]
            n_ocb, cob = pl["n_ocb"], pl["cob"]
            o_start, o_rows = pl["o_start"], pl["o_rows"]
            pairs = o_rows // 2

            prm_sb = pool_st.tile([128, n_ocb, 4], F32, name=f"prm{l}", tag="prm")
            nc.sync.dma_start(out=prm_sb, in_=p_in[l][:])
            wtiles = []
            for ocb in range(n_ocb):
                wshape = [9, 1, 64] if l == 0 else [cib, n_icb * taps, cob]
                wt = pool_w.tile(wshape, F32, name=f"w{l}_{ocb}", tag="w")
                nc.sync.dma_start(out=wt.bitcast(F32R),
                                  in_=w_in[l][ocb].bitcast(F32R))
                wtiles.append(wt)

            act_cur = pool_dr.tile([n_ocb, cob, RT, COLS], F32,
                                   name=f"act{l}", tag="act")
            for ocb in range(n_ocb):
                nc.sync.dma_start(out=act_cur[ocb][:, :, 0:1],
                                  in_=zero_sb[:cob, 0:RT])
                nc.sync.dma_start(out=act_cur[ocb][:, :, COLS - 1:COLS],
                                  in_=zero_sb[:cob, 0:RT])
            rec = [pool_rc.tile([cob, OWN, 6], F32, name=f"rec{l}_{o}", tag="rec")
                   for o in range(n_ocb)]
            stats_sb = pool_st.tile([128, 8], F32, name=f"stats{l}", tag="stats")
            nc.vector.memset(stats_sb, 0.0)

            groups = [(o_start + 2 * GP * g,
                       min(o_start + 2 * GP * (g + 1), o_start + o_rows))
                      for g in range((pairs + GP - 1) // GP)]

            for (r_lo, r_hi) in groups:
                in_tiles = []
                if l == 0:
                    nin = (r_hi - r_lo) * COLS
                    t = pool_in.tile([9, INTILE], F32,
                                     name=f"in0_{r_lo}", tag="in")
                    nc.sync.dma_start(
                        out=t[:, 0:nin].bitcast(F32R),
                        in_=x9_in[:, r_lo * COLS: r_hi * COLS].bitcast(F32R))
                    in_tiles.append(t)
                    in_lo = r_lo
                else:
                    in_lo = r_lo - 1 if k == 3 else r_lo
                    in_hi = r_hi + 1 if k == 3 else r_hi
                    nin = (in_hi - in_lo) * COLS
                    for icb in range(n_icb):
                        t = pool_in.tile([cib, INTILE], F32,
                                         name=f"in{l}_{r_lo}_{icb}", tag="in")
                        src = act_prev[icb].rearrange("c r w -> c (r w)")
                        nc.sync.dma_start(
                            out=t[:, 0:nin + 2].bitcast(F32R),
                            in_=src[:, in_lo * COLS - 1: in_hi * COLS + 1]
                            .bitcast(F32R))
                        dv = t[:, 1:1 + nin].rearrange(
                            "c (r w) -> c r w", w=COLS)[:, :, 1:225]
                        nc.scalar.activation(
                            dv.bitcast(F32R), dv,
                            AF.Relu, bias=b_prev[:cib, icb:icb + 1],
                            scale=a_prev[:cib, icb:icb + 1])
                        if k == 3:
                            # zero rows outside the global image (boundary cores)
                            for (ga, gb, col) in ((0, 6, 0), (34, RT, 1)):
                                a0, b0 = max(in_lo, ga), min(in_hi, gb)
                                if a0 < b0:
                                    sl = t[:, 1 + (a0 - in_lo) * COLS:
                                           1 + (b0 - in_lo) * COLS]
                                    nc.vector.tensor_scalar_mul(
                                        sl.bitcast(F32R), sl,
                                        mask_sb[:cib, col:col + 1])
                        in_tiles.append(t)

                for ocb in range(n_ocb):
                    for r in range(r_lo, r_hi, 2):
                        ps = pool_ps.tile([cob, NPIX], F32, name=f"ps{l}",
                                          tag="ps")
                        if l == 0:
                            rhs = in_tiles[0][:, (r - r_lo) * COLS:
                                              (r - r_lo) * COLS + NPIX]
                            nc.tensor.matmul(ps, wtiles[0][:, 0, :].bitcast(F32R),
                                             rhs.bitcast(F32R),
                                             start=True, stop=True)
                        else:
                            nmm = n_icb * taps
                            i = 0
                            for icb in range(n_icb):
                                for t_i in range(taps):
                                    if k == 3:
                                        ky, kx = t_i // 3, t_i % 3
                                        off = (1 + (r + ky - 1 - in_lo) * COLS
                                               + kx - 1)
                                    else:
                                        off = 1 + (r - in_lo) * COLS
                                    rhs = in_tiles[icb][:, off: off + NPIX]
                                    nc.tensor.matmul(
                                        ps,
                                        wtiles[ocb][:, icb * taps + t_i, :]
                                        .bitcast(F32R),
                                        rhs.bitcast(F32R),
                                        start=(i == 0), stop=(i == nmm - 1))
                                    i += 1
                        ev = pool_ev.tile([cob, NPIX], F32, name=f"ev{l}",
                                          tag="ev")
                        nc.scalar.activation(ev, ps, AF.Identity,
                                             bias=prm_sb[:cob, ocb, 0:1],
                                             scale=1.0)
                        evv = ev.rearrange("c (r w) -> c r w", w=COLS)
                        nc.sync.dma_start(
                            out=act_cur[ocb][:, r:r + 2, 1:225],
                            in_=evv[:, :, 1:225])
                        for rr in (r, r + 1):
                            if 6 <= rr < 34:
                                nc.vector.bn_stats(
                                    rec[ocb][:, rr - 6, :],
                                    ev[:, (rr - r) * COLS + 1:
                                       (rr - r) * COLS + 225])

            # ---- BN stats: aggregate, AllReduce, affine coefficients ----
            mvs = pool_st.tile([cob, n_ocb, 2], F32, name=f"mv{l}", tag="mv")
            tmp = pool_st.tile([128, n_ocb], F32, name=f"tmp{l}", tag="tmp")
            for ocb in range(n_ocb):
                nc.vector.bn_aggr(mvs[:, ocb, :], rec[ocb])
                nc.gpsimd.tensor_copy(stats_sb[:cob, ocb:ocb + 1],
                                      mvs[:, ocb, 0:1])
                nc.vector.tensor_mul(tmp[:cob, 0:1], mvs[:, ocb, 0:1],
                                     mvs[:, ocb, 0:1])
                nc.vector.tensor_add(stats_sb[:cob, n_ocb + ocb:n_ocb + ocb + 1],
                                     tmp[:cob, 0:1], mvs[:, ocb, 1:2])
            ar_i = pool_dr.tile([128, 8], F32, name=f"ari{l}", tag="ari")
            ar_o = pool_sh.tile([128, 8], F32, name=f"aro{l}", tag="aro",
                                addr_space="Shared")
            nc.sync.dma_start(out=ar_i, in_=stats_sb)
            nc.gpsimd.collective_compute(
                "AllReduce", mybir.AluOpType.add,
                replica_groups=[list(range(NCORE))],
                ins=[ar_i.opt()], outs=[ar_o.opt()])
            ar_sb = pool_st.tile([128, 8], F32, name=f"ar{l}", tag="ar")
            nc.sync.dma_start(out=ar_sb, in_=ar_o)

            a_t = pool_st.tile([128, n_ocb], F32, name=f"a{l}", tag="a")
            b_t = pool_st.tile([128, n_ocb], F32, name=f"b{l}", tag="b")
            m_t = pool_st.tile([128, n_ocb], F32, name=f"m{l}", tag="m")
            v_t = pool_st.tile([128, n_ocb], F32, name=f"v{l}", tag="v")
            n_o = n_ocb
            nc.vector.tensor_scalar_mul(m_t[:cob], ar_sb[:cob, 0:n_o], 1.0 / NCORE)
            nc.vector.tensor_scalar_mul(v_t[:cob], ar_sb[:cob, n_o:2 * n_o],
                                        1.0 / NCORE)
            nc.vector.tensor_mul(b_t[:cob], m_t[:cob], m_t[:cob])
            nc.vector.tensor_sub(v_t[:cob], v_t[:cob], b_t[:cob])
            nc.scalar.activation(v_t[:cob], v_t[:cob], AF.Sqrt, bias=eps_sb[:cob])
            nc.vector.reciprocal(v_t[:cob], v_t[:cob])
            nc.vector.tensor_mul(a_t[:cob], v_t[:cob], prm_sb[:cob, :, 1])
            nc.vector.tensor_mul(b_t[:cob], a_t[:cob], m_t[:cob])
            nc.vector.tensor_sub(b_t[:cob], prm_sb[:cob, :, 2], b_t[:cob])

            act_prev, a_prev, b_prev = act_cur, a_t, b_t

        if not full:   # debug: dump raw last tile + affine coefficients
            nc.sync.dma_start(out=xo[:], in_=act_prev[:])
            dbg = pool_st.tile([128, 16], F32, name="dbg", tag="dbg")
            nc.vector.memset(dbg, 0.0)
            nc.vector.tensor_copy(dbg[:last["cob"], 0:last["n_ocb"]],
                                  a_prev[:last["cob"]])
            nc.vector.tensor_copy(dbg[:last["cob"], 8:8 + last["n_ocb"]],
                                  b_prev[:last["cob"]])
            nc.sync.dma_start(out=yo[:], in_=dbg)
        else:
            # ---- final: normalize L14 output, emit x and luminance y ----
            for (r_lo, r_hi) in [(6, 14), (14, 22), (22, 30), (30, 34)]:
                nin = (r_hi - r_lo) * COLS
                t = pool_in.tile([3, INTILE], F32, name=f"fin{r_lo}", tag="in")
                src = act_prev[0].rearrange("c r w -> c (r w)")
                nc.sync.dma_start(
                    out=t[:, 0:nin + 2].bitcast(F32R),
                    in_=src[:, r_lo * COLS - 1: r_hi * COLS + 1].bitcast(F32R))
                dv = t[:, 1:1 + nin].rearrange(
                    "c (r w) -> c r w", w=COLS)[:, :, 1:225]
                nc.scalar.activation(dv.bitcast(F32R), dv, AF.Relu,
                                     bias=b_prev[:3, 0:1], scale=a_prev[:3, 0:1])
                v = t[:, 1:1 + nin].rearrange("c (r w) -> c r w", w=COLS)
                nc.sync.dma_start(out=xo[:, r_lo - 6:r_hi - 6, :],
                                  in_=v[:, :, 1:225])
                for r in range(r_lo, r_hi, 2):
                    ps = pool_ps.tile([1, NPIX], F32, name="psl", tag="ps")
                    off = 1 + (r - r_lo) * COLS
                    nc.tensor.matmul(ps, lum_sb.bitcast(F32R),
                                     t[:, off:off + NPIX].bitcast(F32R),
                                     start=True, stop=True)
                    ev = pool_ev.tile([1, NPIX], F32, name="evl", tag="ev")
                    nc.scalar.activation(ev, ps, AF.Identity, bias=0.0, scale=1.0)
                    vv = ev.rearrange("c (r w) -> c r w", w=COLS)
                    nc.sync.dma_start(out=yo[:, r - 6:r - 6 + 2, :],
                                      in_=vv[:, :, 1:225])

    nc.compile()
    return nc, plans


def _prep_inputs(x, params, n_layers):
    """Per-core input maps. x: (1,1,224,224); params: list of (w,b,g,be)."""
    plans = _plan(n_layers)
    x = np.asarray(x, np.float32)[0, 0]
    shared = {}
    for pl in plans:
        l = pl["l"]
        Wt = np.asarray(params[l][0], np.float32)
        k, taps = pl["k"], pl["taps"]
        n_icb, cib, n_ocb, cob = pl["n_icb"], pl["cib"], pl["n_ocb"], pl["cob"]
        if l == 0:
            w_np = np.zeros((1, 9, 1, 64), np.float32)
            for ky in range(3):
                for kx in range(3):
                    w_np[0, ky * 3 + kx, 0, :] = Wt[:, 0, ky, kx]
        else:
            w_np = np.zeros((n_ocb, cib, n_icb * taps, cob), np.float32)
            for ocb in range(n_ocb):
                for icb in range(n_icb):
                    for t in range(taps):
                        ky, kx = (t // k, t % k) if k == 3 else (0, 0)
                        w_np[ocb, :, icb * taps + t, :] = \
                            Wt[ocb * cob:(ocb + 1) * cob,
                               icb * cib:(icb + 1) * cib, ky, kx].T
        shared[f"w{l}"] = w_np
        p_np = np.zeros((n_ocb, 128, 4), np.float32)
        for ocb in range(n_ocb):
            sl = slice(ocb * cob, (ocb + 1) * cob)
            p_np[ocb, :cob, 0] = np.asarray(params[l][1], np.float32)[sl]
            p_np[ocb, :cob, 1] = np.asarray(params[l][2], np.float32)[sl]
            p_np[ocb, :cob, 2] = np.asarray(params[l][3], np.float32)[sl]
        shared[f"p{l}"] = p_np
    shared["lumw"] = np.array(LUMW, np.float32).reshape(3, 1)

    in_maps = []
    for c in range(NCORE):
        g0 = c * OWN - 6
        xe = np.zeros((RT, W), np.float32)
        for r in range(RT):
            gr = g0 + r
            if 0 <= gr < H:
                xe[r] = x[gr]
        xp = np.zeros((RT + 2, W + 4), np.float32)
        xp[1:RT + 1, 2:W + 2] = xe
        x9 = np.zeros((9, RT * COLS), np.float32)
        for ky in range(3):
            for kx in range(3):
                x9[ky * 3 + kx] = xp[ky:ky + RT, kx:kx + COLS].reshape(-1)
        m = dict(shared)
        m["x9"] = x9
        m["mask"] = np.stack([
            np.full(128, 0.0 if c == 0 else 1.0, np.float32),
            np.full(128, 0.0 if c == NCORE - 1 else 1.0, np.float32)], axis=1)
        in_maps.append(m)
    return in_maps


_CACHE = {}


def _get_program(n_layers):
    if n_layers not in _CACHE:
        _CACHE[n_layers] = build_program(n_layers)
    return _CACHE[n_layers]


def run_cores(x, params, n_layers=len(LAYERS), trace=False):
    nc, _ = _get_program(n_layers)
    in_maps = _prep_inputs(x, params, n_layers)
    return run_bass_kernel_spmd(nc, in_maps, list(range(NCORE)), trace=trace)


def kernel(x, y, params):
    res = run_cores(x, params)
    x_full = np.concatenate([r["xo"] for r in res.results], axis=1)[None]
    y_full = np.concatenate([r["yo"][0] for r in res.results], axis=0)[None, None]
    return x_full.astype(np.float32), y_full.astype(np.float32)


# revision 6
# speedup vs baseline: 1.0445x; 1.0445x over previous
"""ColorNet (15x conv+BN+ReLU, 224x224) on 8 TRN2 NeuronCores.

Strategy: spatial H-sharding, 28 owned rows/core plus a 6-row shrinking
halo (one row per 3x3 conv) so no activation halo exchange is ever
needed; only BN batch-stats cross the cores (one small AllReduce per
layer).  Convs run as fp32r matmuls (full PE rate, ~1e-4 rel err)
accumulating 9 taps x Cin-blocks in PSUM; activations stream through
DRAM between layers with BN+ReLU fused into the next layer's load.
"""
import sys

sys.path.insert(0, "/opt/trn_rl_repo")

from contextlib import ExitStack

import numpy as np

import concourse.bacc as bacc
import concourse.tile as tile
from concourse import mybir
from concourse.bass_utils import run_bass_kernel_spmd

F32 = mybir.dt.float32
F32R = mybir.dt.float32r
AF = mybir.ActivationFunctionType

LAYERS = [(1, 64, 3), (64, 128, 1), (128, 128, 3),
          (128, 256, 1), (256, 256, 3), (256, 256, 1), (256, 256, 3),
          (256, 512, 1), (512, 512, 3), (512, 512, 1), (512, 512, 3),
          (512, 256, 1), (256, 128, 1), (128, 64, 1), (64, 3, 1)]
EPS = 1e-5
H = W = 224
NCORE = 8
OWN = 28            # owned output rows per core
RT = 40             # extended tile rows (6 + 28 + 6)
COLS = 226          # padded row width (1 + 224 + 1)
GP = 4              # row-pairs per load group
NPIX = 452          # matmul moving free size = 2 rows x 226
INTILE = 2 * GP * COLS + 2 * COLS + 2   # load-tile elems (GP pairs + halo + guards)
LUMW = (0.2125, 0.7154, 0.0721)


def _blocks(c):
    if c <= 128:
        return 1, c
    assert c % 128 == 0
    return c // 128, 128


def _plan(n_layers):
    plans = []
    m = 6
    for l in range(n_layers):
        cin, cout, k = LAYERS[l]
        m_in, m_out = m, m - (1 if k == 3 else 0)
        m = m_out
        n_icb, cib = _blocks(cin)
        n_ocb, cob = _blocks(cout)
        plans.append(dict(l=l, k=k, cin=cin, cout=cout, taps=k * k,
                          m_in=m_in, m_out=m_out, n_icb=n_icb, cib=cib,
                          n_ocb=n_ocb, cob=cob, o_start=6 - m_out,
                          o_rows=OWN + 2 * m_out))
    return plans


def build_program(n_layers):
    plans = _plan(n_layers)
    nc = bacc.Bacc(num_devices=NCORE)

    x9_in = nc.declare_dram_parameter("x9", [9, RT * COLS], F32, isOutput=False)
    w_in, p_in = [], []
    for pl in plans:
        l = pl["l"]
        wshape = ([1, 9, 1, 64] if l == 0 else
                  [pl["n_ocb"], pl["cib"], pl["n_icb"] * pl["taps"], pl["cob"]])
        w_in.append(nc.declare_dram_parameter(f"w{l}", wshape, F32, isOutput=False))
        p_in.append(nc.declare_dram_parameter(f"p{l}", [pl["n_ocb"], 128, 4], F32,
                                              isOutput=False))
    mask_in = nc.declare_dram_parameter("mask", [128, 2], F32, isOutput=False)
    lum_in = nc.declare_dram_parameter("lumw", [3, 1], F32, isOutput=False)
    last = plans[-1]
    full = n_layers == len(LAYERS)
    if full:
        xo = nc.declare_dram_parameter("xo", [3, OWN, W], F32, isOutput=True)
        yo = nc.declare_dram_parameter("yo", [1, OWN, W], F32, isOutput=True)
    else:  # debug build: dump last layer's raw (pre-BN) tile + its affine
        xo = nc.declare_dram_parameter(
            "xo", [last["n_ocb"], last["cob"], RT, COLS], F32, isOutput=True)
        yo = nc.declare_dram_parameter("yo", [128, 16], F32, isOutput=True)

    with tile.TileContext(nc) as tc, ExitStack() as ctx:
        pool_w = ctx.enter_context(tc.tile_pool(name="w", bufs=5))
        pool_in = ctx.enter_context(tc.tile_pool(name="in", bufs=6))
        pool_ev = ctx.enter_context(tc.tile_pool(name="ev", bufs=8))
        pool_ps = ctx.enter_context(tc.tile_pool(name="ps", bufs=8, space="PSUM"))
        pool_st = ctx.enter_context(tc.tile_pool(name="st", bufs=2))
        pool_rc = ctx.enter_context(tc.tile_pool(name="rc", bufs=6))
        pool_cn = ctx.enter_context(tc.tile_pool(name="cn", bufs=1))
        pool_dr = ctx.enter_context(tc.tile_pool(name="dr", bufs=2, space="DRAM"))
        pool_sh = ctx.enter_context(tc.tile_pool(name="sh", bufs=2, space="DRAM"))

        mask_sb = pool_cn.tile([128, 2], F32)
        nc.sync.dma_start(out=mask_sb, in_=mask_in[:])
        lum_sb = pool_cn.tile([3, 1], F32)
        nc.sync.dma_start(out=lum_sb.bitcast(F32R), in_=lum_in[:].bitcast(F32R))
        eps_sb = pool_cn.tile([128, 1], F32)
        nc.vector.memset(eps_sb, EPS)
        zero_sb = pool_cn.tile([128, RT], F32)
        nc.vector.memset(zero_sb, 0.0)

        act_prev = None          # DRAM tile holding previous layer's raw output
        a_prev = b_prev = None   # BN affine of previous layer's output

        for pl in plans:
            l, k, taps = pl["l"], pl["k"], pl["taps"]
            n_icb, cib = pl["n_icb"], pl["cib"]
            n_ocb, cob = pl["n_ocb"], pl["cob"]
            o_start, o_rows = pl["o_start"], pl["o_rows"]
            pairs = o_rows // 2

            prm_sb = pool_st.tile([128, n_ocb, 4], F32, name=f"prm{l}", tag="prm")
            nc.sync.dma_start(out=prm_sb, in_=p_in[l][:])
            wtiles = []
            for ocb in range(n_ocb):
                wshape = [9, 1, 64] if l == 0 else [cib, n_icb * taps, cob]
                wt = pool_w.tile(wshape, F32, name=f"w{l}_{ocb}", tag="w")
                nc.gpsimd.dma_start(out=wt.bitcast(F32R),
                                    in_=w_in[l][ocb].bitcast(F32R))
                wtiles.append(wt)

            act_cur = pool_dr.tile([n_ocb, cob, RT, COLS], F32,
                                   name=f"act{l}", tag="act")
            for ocb in range(n_ocb):
                nc.gpsimd.dma_start(out=act_cur[ocb][:, :, 0:1],
                                    in_=zero_sb[:cob, 0:RT])
                nc.gpsimd.dma_start(out=act_cur[ocb][:, :, COLS - 1:COLS],
                                    in_=zero_sb[:cob, 0:RT])
            rec = [pool_rc.tile([cob, OWN, 6], F32, name=f"rec{l}_{o}", tag="rec")
                   for o in range(n_ocb)]
            stats_sb = pool_st.tile([128, 8], F32, name=f"stats{l}", tag="stats")
            nc.vector.memset(stats_sb, 0.0)

            groups = [(o_start + 2 * GP * g,
                       min(o_start + 2 * GP * (g + 1), o_start + o_rows))
                      for g in range((pairs + GP - 1) // GP)]

            for (r_lo, r_hi) in groups:
                in_tiles = []
                if l == 0:
                    nin = (r_hi - r_lo) * COLS
                    t = pool_in.tile([9, INTILE], F32,
                                     name=f"in0_{r_lo}", tag="in")
                    nc.sync.dma_start(
                        out=t[:, 0:nin].bitcast(F32R),
                        in_=x9_in[:, r_lo * COLS: r_hi * COLS].bitcast(F32R))
                    in_tiles.append(t)
                    in_lo = r_lo
                else:
                    in_lo = r_lo - 1 if k == 3 else r_lo
                    in_hi = r_hi + 1 if k == 3 else r_hi
                    nin = (in_hi - in_lo) * COLS
                    for icb in range(n_icb):
                        t = pool_in.tile([cib, INTILE], F32,
                                         name=f"in{l}_{r_lo}_{icb}", tag="in")
                        src = act_prev[icb].rearrange("c r w -> c (r w)")
                        nc.sync.dma_start(
                            out=t[:, 0:nin + 2].bitcast(F32R),
                            in_=src[:, in_lo * COLS - 1: in_hi * COLS + 1]
                            .bitcast(F32R))
                        dv = t[:, 1:1 + nin].rearrange(
                            "c (r w) -> c r w", w=COLS)[:, :, 1:225]
                        nc.scalar.activation(
                            dv.bitcast(F32R), dv,
                            AF.Relu, bias=b_prev[:cib, icb:icb + 1],
                            scale=a_prev[:cib, icb:icb + 1])
                        if k == 3:
                            # zero rows outside the global image (boundary cores)
                            for (ga, gb, col) in ((0, 6, 0), (34, RT, 1)):
                                a0, b0 = max(in_lo, ga), min(in_hi, gb)
                                if a0 < b0:
                                    sl = t[:, 1 + (a0 - in_lo) * COLS:
                                           1 + (b0 - in_lo) * COLS]
                                    nc.vector.tensor_scalar_mul(
                                        sl.bitcast(F32R), sl,
                                        mask_sb[:cib, col:col + 1])
                        in_tiles.append(t)

                for ocb in range(n_ocb):
                    for r in range(r_lo, r_hi, 2):
                        ps = pool_ps.tile([cob, NPIX], F32, name=f"ps{l}",
                                          tag="ps")
                        if l == 0:
                            rhs = in_tiles[0][:, (r - r_lo) * COLS:
                                              (r - r_lo) * COLS + NPIX]
                            nc.tensor.matmul(ps, wtiles[0][:, 0, :].bitcast(F32R),
                                             rhs.bitcast(F32R),
                                             start=True, stop=True)
                        else:
                            nmm = n_icb * taps
                            i = 0
                            for icb in range(n_icb):
                                for t_i in range(taps):
                                    if k == 3:
                                        ky, kx = t_i // 3, t_i % 3
                                        off = (1 + (r + ky - 1 - in_lo) * COLS
                                               + kx - 1)
                                    else:
                                        off = 1 + (r - in_lo) * COLS
                                    rhs = in_tiles[icb][:, off: off + NPIX]
                                    nc.tensor.matmul(
                                        ps,
                                        wtiles[ocb][:, icb * taps + t_i, :]
                                        .bitcast(F32R),
                                        rhs.bitcast(F32R),
                                        start=(i == 0), stop=(i == nmm - 1))
                                    i += 1
                        ev = pool_ev.tile([cob, NPIX], F32, name=f"ev{l}",
                                          tag="ev")
                        nc.vector.tensor_scalar_add(ev, ps,
                                                    prm_sb[:cob, ocb, 0:1])
                        evv = ev.rearrange("c (r w) -> c r w", w=COLS)
                        nc.gpsimd.dma_start(
                            out=act_cur[ocb][:, r:r + 2, 1:225],
                            in_=evv[:, :, 1:225])
                        for rr in (r, r + 1):
                            if 6 <= rr < 34:
                                nc.vector.bn_stats(
                                    rec[ocb][:, rr - 6, :],
                                    ev[:, (rr - r) * COLS + 1:
                                       (rr - r) * COLS + 225])

            # ---- BN stats: aggregate, AllReduce, affine coefficients ----
            mvs = pool_st.tile([cob, n_ocb, 2], F32, name=f"mv{l}", tag="mv")
            tmp = pool_st.tile([128, n_ocb], F32, name=f"tmp{l}", tag="tmp")
            for ocb in range(n_ocb):
                nc.vector.bn_aggr(mvs[:, ocb, :], rec[ocb])
                nc.gpsimd.tensor_copy(stats_sb[:cob, ocb:ocb + 1],
                                      mvs[:, ocb, 0:1])
                nc.vector.tensor_mul(tmp[:cob, 0:1], mvs[:, ocb, 0:1],
                                     mvs[:, ocb, 0:1])
                nc.vector.tensor_add(stats_sb[:cob, n_ocb + ocb:n_ocb + ocb + 1],
                                     tmp[:cob, 0:1], mvs[:, ocb, 1:2])
            ar_i = pool_dr.tile([128, 8], F32, name=f"ari{l}", tag="ari")
            ar_o = pool_sh.tile([128, 8], F32, name=f"aro{l}", tag="aro",
                                addr_space="Shared")
            nc.sync.dma_start(out=ar_i, in_=stats_sb)
            nc.gpsimd.collective_compute(
                "AllReduce", mybir.AluOpType.add,
                replica_groups=[list(range(NCORE))],
                ins=[ar_i.opt()], outs=[ar_o.opt()])
            ar_sb = pool_st.tile([128, 8], F32, name=f"ar{l}", tag="ar")
            nc.sync.dma_start(out=ar_sb, in_=ar_o)

            a_t = pool_st.tile([128, n_ocb], F32, name=f"a{l}", tag="a")
            b_t = pool_st.tile([128, n_ocb], F32, name=f"b{l}", tag="b")
            m_t = pool_st.tile([128, n_ocb], F32, name=f"m{l}", tag="m")
            v_t = pool_st.tile([128, n_ocb], F32, name=f"v{l}", tag="v")
            n_o = n_ocb
            nc.vector.tensor_scalar_mul(m_t[:cob], ar_sb[:cob, 0:n_o], 1.0 / NCORE)
            nc.vector.tensor_scalar_mul(v_t[:cob], ar_sb[:cob, n_o:2 * n_o],
                                        1.0 / NCORE)
            nc.vector.tensor_mul(b_t[:cob], m_t[:cob], m_t[:cob])
            nc.vector.tensor_sub(v_t[:cob], v_t[:cob], b_t[:cob])
            nc.scalar.activation(v_t[:cob], v_t[:cob], AF.Sqrt, bias=eps_sb[:cob])
            nc.vector.reciprocal(v_t[:cob], v_t[:cob])
            nc.vector.tensor_mul(a_t[:cob], v_t[:cob], prm_sb[:cob, :, 1])
            nc.vector.tensor_mul(b_t[:cob], a_t[:cob], m_t[:cob])
            nc.vector.tensor_sub(b_t[:cob], prm_sb[:cob, :, 2], b_t[:cob])

            act_prev, a_prev, b_prev = act_cur, a_t, b_t

        if not full:   # debug: dump raw last tile + affine coefficients
            nc.sync.dma_start(out=xo[:], in_=act_prev[:])
            dbg = pool_st.tile([128, 16], F32, name="dbg", tag="dbg")
            nc.vector.memset(dbg, 0.0)
            nc.vector.tensor_copy(dbg[:last["cob"], 0:last["n_ocb"]],
                                  a_prev[:last["cob"]])
            nc.vector.tensor_copy(dbg[:last["cob"], 8:8 + last["n_ocb"]],
                                  b_prev[:last["cob"]])
            nc.sync.dma_start(out=yo[:], in_=dbg)
        else:
            # ---- final: normalize L14 output, emit x and luminance y ----
            for (r_lo, r_hi) in [(6, 14), (14, 22), (22, 30), (30, 34)]:
                nin = (r_hi - r_lo) * COLS
                t = pool_in.tile([3, INTILE], F32, name=f"fin{r_lo}", tag="in")
                src = act_prev[0].rearrange("c r w -> c (r w)")
                nc.sync.dma_start(
                    out=t[:, 0:nin + 2].bitcast(F32R),
                    in_=src[:, r_lo * COLS - 1: r_hi * COLS + 1].bitcast(F32R))
                dv = t[:, 1:1 + nin].rearrange(
                    "c (r w) -> c r w", w=COLS)[:, :, 1:225]
                nc.scalar.activation(dv.bitcast(F32R), dv, AF.Relu,
                                     bias=b_prev[:3, 0:1], scale=a_prev[:3, 0:1])
                v = t[:, 1:1 + nin].rearrange("c (r w) -> c r w", w=COLS)
                nc.sync.dma_start(out=xo[:, r_lo - 6:r_hi - 6, :],
                                  in_=v[:, :, 1:225])
                for r in range(r_lo, r_hi, 2):
                    ps = pool_ps.tile([1, NPIX], F32, name="psl", tag="ps")
                    off = 1 + (r - r_lo) * COLS
                    nc.tensor.matmul(ps, lum_sb.bitcast(F32R),
                                     t[:, off:off + NPIX].bitcast(F32R),
                                     start=True, stop=True)
                    ev = pool_ev.tile([1, NPIX], F32, name="evl", tag="ev")
                    nc.vector.tensor_copy(ev, ps)
                    vv = ev.rearrange("c (r w) -> c r w", w=COLS)
                    nc.sync.dma_start(out=yo[:, r - 6:r - 6 + 2, :],
                                      in_=vv[:, :, 1:225])

    nc.compile()
    return nc, plans


def _prep_inputs(x, params, n_layers):
    """Per-core input maps. x: (1,1,224,224); params: list of (w,b,g,be)."""
    plans = _plan(n_layers)
    x = np.asarray(x, np.float32)[0, 0]
    shared = {}
    for pl in plans:
        l = pl["l"]
        Wt = np.asarray(params[l][0], np.float32)
        k, taps = pl["k"], pl["taps"]
        n_icb, cib, n_ocb, cob = pl["n_icb"], pl["cib"], pl["n_ocb"], pl["cob"]
        if l == 0:
            w_np = np.zeros((1, 9, 1, 64), np.float32)
            for ky in range(3):
                for kx in range(3):
                    w_np[0, ky * 3 + kx, 0, :] = Wt[:, 0, ky, kx]
        else:
            w_np = np.zeros((n_ocb, cib, n_icb * taps, cob), np.float32)
            for ocb in range(n_ocb):
                for icb in range(n_icb):
                    for t in range(taps):
                        ky, kx = (t // k, t % k) if k == 3 else (0, 0)
                        w_np[ocb, :, icb * taps + t, :] = \
                            Wt[ocb * cob:(ocb + 1) * cob,
                               icb * cib:(icb + 1) * cib, ky, kx].T
        shared[f"w{l}"] = w_np
        p_np = np.zeros((n_ocb, 128, 4), np.float32)
        for ocb in range(n_ocb):
            sl = slice(ocb * cob, (ocb + 1) * cob)
            p_np[ocb, :cob, 0] = np.asarray(params[l][1], np.float32)[sl]
            p_np[ocb, :cob, 1] = np.asarray(params[l][2], np.float32)[sl]
            p_np[ocb, :cob, 2] = np.asarray(params[l][3], np.float32)[sl]
        shared[f"p{l}"] = p_np
    shared["lumw"] = np.array(LUMW, np.float32).reshape(3, 1)

    in_maps = []
    for c in range(NCORE):
        g0 = c * OWN - 6
        xe = np.zeros((RT, W), np.float32)
        for r in range(RT):
            gr = g0 + r
            if 0 <= gr < H:
                xe[r] = x[gr]
        xp = np.zeros((RT + 2, W + 4), np.float32)
        xp[1:RT + 1, 2:W + 2] = xe
        x9 = np.zeros((9, RT * COLS), np.float32)
        for ky in range(3):
            for kx in range(3):
                x9[ky * 3 + kx] = xp[ky:ky + RT, kx:kx + COLS].reshape(-1)
        m = dict(shared)
        m["x9"] = x9
        m["mask"] = np.stack([
            np.full(128, 0.0 if c == 0 else 1.0, np.float32),
            np.full(128, 0.0 if c == NCORE - 1 else 1.0, np.float32)], axis=1)
        in_maps.append(m)
    return in_maps


_CACHE = {}


def _get_program(n_layers):
    if n_layers not in _CACHE:
        _CACHE[n_layers] = build_program(n_layers)
    return _CACHE[n_layers]


def run_cores(x, params, n_layers=len(LAYERS), trace=False):
    nc, _ = _get_program(n_layers)
    in_maps = _prep_inputs(x, params, n_layers)
    return run_bass_kernel_spmd(nc, in_maps, list(range(NCORE)), trace=trace)


def kernel(x, y, params):
    res = run_cores(x, params)
    x_full = np.concatenate([r["xo"] for r in res.results], axis=1)[None]
    y_full = np.concatenate([r["yo"][0] for r in res.results], axis=0)[None, None]
    return x_full.astype(np.float32), y_full.astype(np.float32)


# revision 8
# speedup vs baseline: 1.5357x; 1.4702x over previous
"""ColorNet (15x conv+BN+ReLU, 224x224) on 8 TRN2 NeuronCores.

Strategy: spatial H-sharding, 28 owned rows/core plus a 6-row shrinking
halo (one row per 3x3 conv) so no activation halo exchange is ever
needed; only BN batch-stats cross the cores (one small AllReduce per
layer).  Convs run as fp32r matmuls (full PE rate, ~1e-4 rel err)
accumulating 9 taps x Cin-blocks in PSUM; activations stream through
DRAM between layers with BN+ReLU fused into the next layer's load.
"""
import sys

sys.path.insert(0, "/opt/trn_rl_repo")

from contextlib import ExitStack

import numpy as np

import concourse.bacc as bacc
import concourse.tile as tile
from concourse import mybir
from concourse.bass_utils import run_bass_kernel_spmd

F32 = mybir.dt.float32
F32R = mybir.dt.float32r
AF = mybir.ActivationFunctionType

LAYERS = [(1, 64, 3), (64, 128, 1), (128, 128, 3),
          (128, 256, 1), (256, 256, 3), (256, 256, 1), (256, 256, 3),
          (256, 512, 1), (512, 512, 3), (512, 512, 1), (512, 512, 3),
          (512, 256, 1), (256, 128, 1), (128, 64, 1), (64, 3, 1)]
EPS = 1e-5
H = W = 224
NCORE = 8
OWN = 28            # owned output rows per core
RT = 40             # extended tile rows (6 + 28 + 6)
COLS = 226          # padded row width (1 + 224 + 1)
GP = 4              # row-pairs per load group
NPIX = 452          # matmul moving free size = 2 rows x 226
INTILE = 2 * GP * COLS + 2 * COLS + 2   # load-tile elems (GP pairs + halo + guards)
LUMW = (0.2125, 0.7154, 0.0721)


def _blocks(c):
    if c <= 128:
        return 1, c
    assert c % 128 == 0
    return c // 128, 128


def _plan(n_layers):
    plans = []
    m = 6
    for l in range(n_layers):
        cin, cout, k = LAYERS[l]
        m_in, m_out = m, m - (1 if k == 3 else 0)
        m = m_out
        n_icb, cib = _blocks(cin)
        n_ocb, cob = _blocks(cout)
        plans.append(dict(l=l, k=k, cin=cin, cout=cout, taps=k * k,
                          m_in=m_in, m_out=m_out, n_icb=n_icb, cib=cib,
                          n_ocb=n_ocb, cob=cob, o_start=6 - m_out,
                          o_rows=OWN + 2 * m_out))
    return plans


def build_program(n_layers):
    plans = _plan(n_layers)
    nc = bacc.Bacc(num_devices=NCORE)

    x9_in = nc.declare_dram_parameter("x9", [9, RT * COLS], F32, isOutput=False)
    w_in, p_in = [], []
    for pl in plans:
        l = pl["l"]
        wshape = ([1, 9, 1, 64] if l == 0 else
                  [pl["n_ocb"], pl["cib"], pl["n_icb"] * pl["taps"], pl["cob"]])
        w_in.append(nc.declare_dram_parameter(f"w{l}", wshape, F32, isOutput=False))
        p_in.append(nc.declare_dram_parameter(f"p{l}", [pl["n_ocb"], 128, 4], F32,
                                              isOutput=False))
    mask_in = nc.declare_dram_parameter("mask", [128, 2], F32, isOutput=False)
    lum_in = nc.declare_dram_parameter("lumw", [3, 1], F32, isOutput=False)
    last = plans[-1]
    full = n_layers == len(LAYERS)
    if full:
        xo = nc.declare_dram_parameter("xo", [3, OWN, W], F32, isOutput=True)
        yo = nc.declare_dram_parameter("yo", [1, OWN, W], F32, isOutput=True)
    else:  # debug build: dump last layer's raw (pre-BN) tile + its affine
        xo = nc.declare_dram_parameter(
            "xo", [last["n_ocb"], last["cob"], RT, COLS], F32, isOutput=True)
        yo = nc.declare_dram_parameter("yo", [128, 16], F32, isOutput=True)

    with tile.TileContext(nc) as tc, ExitStack() as ctx:
        pool_w = ctx.enter_context(tc.tile_pool(name="w", bufs=5))
        pool_in = ctx.enter_context(tc.tile_pool(name="in", bufs=6))
        pool_ev = ctx.enter_context(tc.tile_pool(name="ev", bufs=8))
        pool_ps = ctx.enter_context(tc.tile_pool(name="ps", bufs=8, space="PSUM"))
        pool_st = ctx.enter_context(tc.tile_pool(name="st", bufs=2))
        pool_rc = ctx.enter_context(tc.tile_pool(name="rc", bufs=6))
        pool_cn = ctx.enter_context(tc.tile_pool(name="cn", bufs=1))
        pool_dr = ctx.enter_context(tc.tile_pool(name="dr", bufs=2, space="DRAM"))
        pool_sh = ctx.enter_context(tc.tile_pool(name="sh", bufs=2, space="DRAM"))

        mask_sb = pool_cn.tile([128, 2], F32)
        nc.sync.dma_start(out=mask_sb, in_=mask_in[:])
        lum_sb = pool_cn.tile([3, 1], F32)
        nc.sync.dma_start(out=lum_sb.bitcast(F32R), in_=lum_in[:].bitcast(F32R))
        eps_sb = pool_cn.tile([128, 1], F32)
        nc.vector.memset(eps_sb, EPS)
        zero_sb = pool_cn.tile([128, RT], F32)
        nc.vector.memset(zero_sb, 0.0)

        act_prev = None          # DRAM tile holding previous layer's raw output
        a_prev = b_prev = None   # BN affine of previous layer's output

        for pl in plans:
            l, k, taps = pl["l"], pl["k"], pl["taps"]
            n_icb, cib = pl["n_icb"], pl["cib"]
            n_ocb, cob = pl["n_ocb"], pl["cob"]
            o_start, o_rows = pl["o_start"], pl["o_rows"]
            pairs = o_rows // 2

            prm_sb = pool_st.tile([128, n_ocb, 4], F32, name=f"prm{l}", tag="prm")
            nc.sync.dma_start(out=prm_sb, in_=p_in[l][:])
            wtiles = []
            for ocb in range(n_ocb):
                wshape = [9, 1, 64] if l == 0 else [cib, n_icb * taps, cob]
                wt = pool_w.tile(wshape, F32, name=f"w{l}_{ocb}", tag="w")
                nc.gpsimd.dma_start(out=wt.bitcast(F32R),
                                    in_=w_in[l][ocb].bitcast(F32R))
                wtiles.append(wt)

            act_cur = pool_dr.tile([n_ocb, cob, RT, W], F32,
                                   name=f"act{l}", tag="act")
            rec = [pool_rc.tile([cob, OWN, 6], F32, name=f"rec{l}_{o}", tag="rec")
                   for o in range(n_ocb)]
            stats_sb = pool_st.tile([128, 8], F32, name=f"stats{l}", tag="stats")
            nc.vector.memset(stats_sb, 0.0)

            groups = [(o_start + 2 * GP * g,
                       min(o_start + 2 * GP * (g + 1), o_start + o_rows))
                      for g in range((pairs + GP - 1) // GP)]

            for (r_lo, r_hi) in groups:
                in_tiles = []
                if l == 0:
                    nin = (r_hi - r_lo) * COLS
                    t = pool_in.tile([9, INTILE], F32,
                                     name=f"in0_{r_lo}", tag="in")
                    nc.sync.dma_start(
                        out=t[:, 0:nin].bitcast(F32R),
                        in_=x9_in[:, r_lo * COLS: r_hi * COLS].bitcast(F32R))
                    in_tiles.append(t)
                    in_lo = r_lo
                else:
                    in_lo = r_lo - 1 if k == 3 else r_lo
                    in_hi = r_hi + 1 if k == 3 else r_hi
                    nin = (in_hi - in_lo) * COLS
                    for icb in range(n_icb):
                        t = pool_in.tile([cib, INTILE], F32,
                                         name=f"in{l}_{r_lo}_{icb}", tag="in")
                        dv = t[:, 1:1 + nin].rearrange(
                            "c (r w) -> c r w", w=COLS)[:, :, 1:225]
                        nc.sync.dma_start(
                            out=dv.bitcast(F32R),
                            in_=act_prev[icb][:, in_lo:in_hi, :].bitcast(F32R))
                        nc.scalar.activation(
                            dv.bitcast(F32R), dv,
                            AF.Relu, bias=b_prev[:cib, icb:icb + 1],
                            scale=a_prev[:cib, icb:icb + 1])
                        if k == 3:
                            # zero the pad/guard columns (slot holds stale data)
                            zv = zero_sb[:cib, 0:40].rearrange(
                                "c (r w) -> c r w", w=2)[:, 0:10, :]
                            pv = t[:, 0:2260].rearrange(
                                "c (r w) -> c r w", w=COLS)[:, :, 0:2]
                            nc.vector.tensor_copy(pv.bitcast(F32R), zv)
                            nc.vector.tensor_copy(
                                t[:, 2260:2262].bitcast(F32R),
                                zero_sb[:cib, 0:2])
                            # zero rows outside the global image (boundary cores)
                            for (ga, gb, col) in ((0, 6, 0), (34, RT, 1)):
                                a0, b0 = max(in_lo, ga), min(in_hi, gb)
                                if a0 < b0:
                                    sl = t[:, 1 + (a0 - in_lo) * COLS:
                                           1 + (b0 - in_lo) * COLS]
                                    nc.vector.tensor_scalar_mul(
                                        sl.bitcast(F32R), sl,
                                        mask_sb[:cib, col:col + 1])
                        in_tiles.append(t)

                for ocb in range(n_ocb):
                    for r in range(r_lo, r_hi, 2):
                        ps = pool_ps.tile([cob, NPIX], F32, name=f"ps{l}",
                                          tag="ps")
                        if l == 0:
                            rhs = in_tiles[0][:, (r - r_lo) * COLS:
                                              (r - r_lo) * COLS + NPIX]
                            nc.tensor.matmul(ps, wtiles[0][:, 0, :].bitcast(F32R),
                                             rhs.bitcast(F32R),
                                             start=True, stop=True)
                        else:
                            nmm = n_icb * taps
                            i = 0
                            for icb in range(n_icb):
                                for t_i in range(taps):
                                    if k == 3:
                                        ky, kx = t_i // 3, t_i % 3
                                        off = (1 + (r + ky - 1 - in_lo) * COLS
                                               + kx - 1)
                                    else:
                                        off = 1 + (r - in_lo) * COLS
                                    rhs = in_tiles[icb][:, off: off + NPIX]
                                    nc.tensor.matmul(
                                        ps,
                                        wtiles[ocb][:, icb * taps + t_i, :]
                                        .bitcast(F32R),
                                        rhs.bitcast(F32R),
                                        start=(i == 0), stop=(i == nmm - 1))
                                    i += 1
                        ev = pool_ev.tile([cob, NPIX], F32, name=f"ev{l}",
                                          tag="ev")
                        nc.vector.tensor_scalar_add(ev, ps,
                                                    prm_sb[:cob, ocb, 0:1])
                        evv = ev.rearrange("c (r w) -> c r w", w=COLS)
                        nc.gpsimd.dma_start(
                            out=act_cur[ocb][:, r:r + 2, :],
                            in_=evv[:, :, 1:225])
                        for rr in (r, r + 1):
                            if 6 <= rr < 34:
                                nc.vector.bn_stats(
                                    rec[ocb][:, rr - 6, :],
                                    ev[:, (rr - r) * COLS + 1:
                                       (rr - r) * COLS + 225])

            # ---- BN stats: aggregate, AllReduce, affine coefficients ----
            mvs = pool_st.tile([cob, n_ocb, 2], F32, name=f"mv{l}", tag="mv")
            tmp = pool_st.tile([128, n_ocb], F32, name=f"tmp{l}", tag="tmp")
            for ocb in range(n_ocb):
                nc.vector.bn_aggr(mvs[:, ocb, :], rec[ocb])
                nc.gpsimd.tensor_copy(stats_sb[:cob, ocb:ocb + 1],
                                      mvs[:, ocb, 0:1])
                nc.vector.tensor_mul(tmp[:cob, 0:1], mvs[:, ocb, 0:1],
                                     mvs[:, ocb, 0:1])
                nc.vector.tensor_add(stats_sb[:cob, n_ocb + ocb:n_ocb + ocb + 1],
                                     tmp[:cob, 0:1], mvs[:, ocb, 1:2])
            ar_i = pool_dr.tile([128, 8], F32, name=f"ari{l}", tag="ari")
            ar_o = pool_sh.tile([128, 8], F32, name=f"aro{l}", tag="aro",
                                addr_space="Shared")
            nc.sync.dma_start(out=ar_i, in_=stats_sb)
            nc.gpsimd.collective_compute(
                "AllReduce", mybir.AluOpType.add,
                replica_groups=[list(range(NCORE))],
                ins=[ar_i.opt()], outs=[ar_o.opt()])
            ar_sb = pool_st.tile([128, 8], F32, name=f"ar{l}", tag="ar")
            nc.sync.dma_start(out=ar_sb, in_=ar_o)

            a_t = pool_st.tile([128, n_ocb], F32, name=f"a{l}", tag="a")
            b_t = pool_st.tile([128, n_ocb], F32, name=f"b{l}", tag="b")
            m_t = pool_st.tile([128, n_ocb], F32, name=f"m{l}", tag="m")
            v_t = pool_st.tile([128, n_ocb], F32, name=f"v{l}", tag="v")
            n_o = n_ocb
            nc.vector.tensor_scalar_mul(m_t[:cob], ar_sb[:cob, 0:n_o], 1.0 / NCORE)
            nc.vector.tensor_scalar_mul(v_t[:cob], ar_sb[:cob, n_o:2 * n_o],
                                        1.0 / NCORE)
            nc.vector.tensor_mul(b_t[:cob], m_t[:cob], m_t[:cob])
            nc.vector.tensor_sub(v_t[:cob], v_t[:cob], b_t[:cob])
            nc.scalar.activation(v_t[:cob], v_t[:cob], AF.Sqrt, bias=eps_sb[:cob])
            nc.vector.reciprocal(v_t[:cob], v_t[:cob])
            nc.vector.tensor_mul(a_t[:cob], v_t[:cob], prm_sb[:cob, :, 1])
            nc.vector.tensor_mul(b_t[:cob], a_t[:cob], m_t[:cob])
            nc.vector.tensor_sub(b_t[:cob], prm_sb[:cob, :, 2], b_t[:cob])

            act_prev, a_prev, b_prev = act_cur, a_t, b_t

        if not full:   # debug: dump raw last tile + affine coefficients
            nc.sync.dma_start(out=xo[:], in_=act_prev[:])
            dbg = pool_st.tile([128, 16], F32, name="dbg", tag="dbg")
            nc.vector.memset(dbg, 0.0)
            nc.vector.tensor_copy(dbg[:last["cob"], 0:last["n_ocb"]],
                                  a_prev[:last["cob"]])
            nc.vector.tensor_copy(dbg[:last["cob"], 8:8 + last["n_ocb"]],
                                  b_prev[:last["cob"]])
            nc.sync.dma_start(out=yo[:], in_=dbg)
        else:
            # ---- final: normalize L14 output, emit x and luminance y ----
            for (r_lo, r_hi) in [(6, 14), (14, 22), (22, 30), (30, 34)]:
                nin = (r_hi - r_lo) * COLS
                t = pool_in.tile([3, INTILE], F32, name=f"fin{r_lo}", tag="in")
                dv = t[:, 1:1 + nin].rearrange(
                    "c (r w) -> c r w", w=COLS)[:, :, 1:225]
                nc.sync.dma_start(
                    out=dv.bitcast(F32R),
                    in_=act_prev[0][:, r_lo:r_hi, :].bitcast(F32R))
                nc.scalar.activation(dv.bitcast(F32R), dv, AF.Relu,
                                     bias=b_prev[:3, 0:1], scale=a_prev[:3, 0:1])
                v = t[:, 1:1 + nin].rearrange("c (r w) -> c r w", w=COLS)
                nc.sync.dma_start(out=xo[:, r_lo - 6:r_hi - 6, :],
                                  in_=v[:, :, 1:225])
                for r in range(r_lo, r_hi, 2):
                    ps = pool_ps.tile([1, NPIX], F32, name="psl", tag="ps")
                    off = 1 + (r - r_lo) * COLS
                    nc.tensor.matmul(ps, lum_sb.bitcast(F32R),
                                     t[:, off:off + NPIX].bitcast(F32R),
                                     start=True, stop=True)
                    ev = pool_ev.tile([1, NPIX], F32, name="evl", tag="ev")
                    nc.vector.tensor_copy(ev, ps)
                    vv = ev.rearrange("c (r w) -> c r w", w=COLS)
                    nc.sync.dma_start(out=yo[:, r - 6:r - 6 + 2, :],
                                      in_=vv[:, :, 1:225])

    nc.compile()
    return nc, plans


def _prep_inputs(x, params, n_layers):
    """Per-core input maps. x: (1,1,224,224); params: list of (w,b,g,be)."""
    plans = _plan(n_layers)
    x = np.asarray(x, np.float32)[0, 0]
    shared = {}
    for pl in plans:
        l = pl["l"]
        Wt = np.asarray(params[l][0], np.float32)
        k, taps = pl["k"], pl["taps"]
        n_icb, cib, n_ocb, cob = pl["n_icb"], pl["cib"], pl["n_ocb"], pl["cob"]
        if l == 0:
            w_np = np.zeros((1, 9, 1, 64), np.float32)
            for ky in range(3):
                for kx in range(3):
                    w_np[0, ky * 3 + kx, 0, :] = Wt[:, 0, ky, kx]
        else:
            w_np = np.zeros((n_ocb, cib, n_icb * taps, cob), np.float32)
            for ocb in range(n_ocb):
                for icb in range(n_icb):
                    for t in range(taps):
                        ky, kx = (t // k, t % k) if k == 3 else (0, 0)
                        w_np[ocb, :, icb * taps + t, :] = \
                            Wt[ocb * cob:(ocb + 1) * cob,
                               icb * cib:(icb + 1) * cib, ky, kx].T
        shared[f"w{l}"] = w_np
        p_np = np.zeros((n_ocb, 128, 4), np.float32)
        for ocb in range(n_ocb):
            sl = slice(ocb * cob, (ocb + 1) * cob)
            p_np[ocb, :cob, 0] = np.asarray(params[l][1], np.float32)[sl]
            p_np[ocb, :cob, 1] = np.asarray(params[l][2], np.float32)[sl]
            p_np[ocb, :cob, 2] = np.asarray(params[l][3], np.float32)[sl]
        shared[f"p{l}"] = p_np
    shared["lumw"] = np.array(LUMW, np.float32).reshape(3, 1)

    in_maps = []
    for c in range(NCORE):
        g0 = c * OWN - 6
        xe = np.zeros((RT, W), np.float32)
        for r in range(RT):
            gr = g0 + r
            if 0 <= gr < H:
                xe[r] = x[gr]
        xp = np.zeros((RT + 2, W + 4), np.float32)
        xp[1:RT + 1, 2:W + 2] = xe
        x9 = np.zeros((9, RT * COLS), np.float32)
        for ky in range(3):
            for kx in range(3):
                x9[ky * 3 + kx] = xp[ky:ky + RT, kx:kx + COLS].reshape(-1)
        m = dict(shared)
        m["x9"] = x9
        m["mask"] = np.stack([
            np.full(128, 0.0 if c == 0 else 1.0, np.float32),
            np.full(128, 0.0 if c == NCORE - 1 else 1.0, np.float32)], axis=1)
        in_maps.append(m)
    return in_maps


_CACHE = {}


def _get_program(n_layers):
    if n_layers not in _CACHE:
        _CACHE[n_layers] = build_program(n_layers)
    return _CACHE[n_layers]


def run_cores(x, params, n_layers=len(LAYERS), trace=False):
    nc, _ = _get_program(n_layers)
    in_maps = _prep_inputs(x, params, n_layers)
    return run_bass_kernel_spmd(nc, in_maps, list(range(NCORE)), trace=trace)


def kernel(x, y, params):
    res = run_cores(x, params)
    x_full = np.concatenate([r["xo"] for r in res.results], axis=1)[None]
    y_full = np.concatenate([r["yo"][0] for r in res.results], axis=0)[None, None]
    return x_full.astype(np.float32), y_full.astype(np.float32)
